# revision 24
# baseline (speedup 1.0000x reference)
"""Trainium2 Bass kernel for CoreferenceResolution.

Math: logits[b,p] = relu(concat(M[b,i], M[b,j], ED[e]) @ W1 + b1) @ W2 + b2
Decomposed as: relu(U[b,i] + V[b,j] + Emeta) @ W2 + b2 with
  U = M @ W1[:768], V = M @ W1[768:1536],
  Emeta[:, p] = W1c^T @ [ed_table[e_p]; 1]   (b1 folded via the ones row).

U/V lookups run on the TensorEngine as one-hot matmuls in a transposed
layout (preH^T[h, pair] accumulated in PSUM). The ED contribution needs no
gather matmul of its own: the host ships the per-pair 25-dim meta vectors
inside rows 96..121 of the V one-hot plane, and the V table's rows 96..121
hold W1c (replicated per chunk), so each V gather matmul applies
W1c^T @ [ed_table[e]; 1] for free (b1 folded via the ones row). relu fuses
into the PSUM drain, alternating ScalarE / VectorE per 3-bank group. All
one-hot planes are precomputed on the HOST and shipped as bf16 (their DMA
overlaps compute; no on-device mask building).

Static structure (8 cores = 2 batches x 4 V-buckets):
 - pairs go to the core owning b's mention chunk-of-512; each core's mention
   table is host-reordered so its V bucket is rows 0..511, laid out in 6
   chunks of 96 rows (rows 96..127 reserved for the W1c meta rows).
 - within a core, columns are statically partitioned into 108 quota cells,
   one per (a-chunk, v-chunk), sized mean + 1.5 sigma, lexicographic
   (a-chunk outer). U and V gather matmuls then cover exact static runs
   (~1.7 U runs, ~5 V runs per 512-col tile). Cell overflow pairs
   (a few dozen per core) are computed on host as fallback.
All inputs ship as a single packed bf16 tensor per core (the per-call
tunnel cost is dominated by tensor COUNT, not bytes). W1 ships full per
core (no collective); mentions ship pre-transposed (no DMA transpose).
"""

import math
import sys

sys.path.insert(0, "/opt/trn_rl_repo")

import numpy as np

HIDDEN = 768
HC = 6                        # hidden chunks of 128
B = 2
N_MENT = 2000
N_PAIRS = 40000
ED_COUNT = 300
META = 25
W1_ROWS_PAD = 1664            # 1561 -> 13 chunks of 128
W1_CHUNKS = 13
N_CORES = 8
SLICES = 4                    # V buckets (of 512 mentions) per batch
T = 512                       # pair columns per tile

N_EXP = 10000                 # mean pairs per core

# mention-table chunking: the 512 bucket mentions live in 6 chunks of 96
# (rows 96..127 of those chunks: 26 rows of W1c meta + zero pad, so the V
# gather matmul applies the ED-meta contribution for free); the remaining
# 1488 mentions use full 128-row chunks.
VROWS = 96                    # one-hot rows per V chunk
V_CHUNKS = 6
REAL_V = [96, 96, 96, 96, 96, 32]
REAL_A = REAL_V + [128] * 11 + [80]
M_CHUNKS = len(REAL_A)        # 18
MENT_PAD = M_CHUNKS * 128     # 2304


def _cell_quotas():
    """Quota per (a-chunk, v-chunk) cell: mean + 1.5 sigma (overflow pairs
    are computed exactly on the host; expected only a few dozen per core)."""
    qs = np.zeros((M_CHUNKS, V_CHUNKS), np.int64)
    for c in range(M_CHUNKS):
        for j in range(V_CHUNKS):
            mean = N_EXP * (REAL_A[c] / N_MENT) * (REAL_V[j] / 512.0)
            qs[c, j] = int(math.ceil(mean + 1.5 * math.sqrt(mean)))
    return qs


CELLQ = _cell_quotas()
_total = int(CELLQ.sum())
NT_ALL = (_total + T - 1) // T
CELLQ[M_CHUNKS - 1, V_CHUNKS - 1] += NT_ALL * T - _total   # pad to tile grid
CELL_BASE = np.zeros((M_CHUNKS, V_CHUNKS), np.int64)
_acc = 0
for _c in range(M_CHUNKS):
    for _j in range(V_CHUNKS):
        CELL_BASE[_c, _j] = _acc
        _acc += CELLQ[_c, _j]


def _runs():
    """Static per-tile U and V matmul runs: lists of (chunk, lo, hi)."""
    uruns = [[] for _ in range(NT_ALL)]
    vruns = [[] for _ in range(NT_ALL)]
    for c in range(M_CHUNKS):
        for j in range(V_CHUNKS):
            g0 = int(CELL_BASE[c, j])
            g1 = g0 + int(CELLQ[c, j])
            t0, t1 = g0 // T, (g1 - 1) // T
            for t in range(t0, t1 + 1):
                lo = max(g0, t * T) - t * T
                hi = min(g1, (t + 1) * T) - t * T
                vruns[t].append((j, lo, hi))
                if uruns[t] and uruns[t][-1][0] == c and uruns[t][-1][2] == lo:
                    uruns[t][-1] = (c, uruns[t][-1][1], hi)
                else:
                    uruns[t].append((c, lo, hi))
    return uruns, vruns


URUNS, VRUNS = _runs()
N_PLANES = 2                  # U one-hot; V one-hot + meta rows

_COMPILED = None

# packed single-input layout (bf16 elements)
OFF_W1 = 0
OFF_MENTS = OFF_W1 + 128 * W1_CHUNKS * HIDDEN
OFF_W2B = OFF_MENTS + 128 * HC * MENT_PAD
OFF_MASKS = OFF_W2B + 128 * HC
OFF_VMETA = OFF_MASKS + 128 * NT_ALL * N_PLANES * T
PACK_TOTAL = OFF_VMETA + 32 * V_CHUNKS * HIDDEN


def _build(phases="pd", reps=1):
    import concourse.mybir as mybir
    import concourse.tile as tile
    from concourse import bacc
    from concourse.bass import ts

    dt = mybir.dt
    nc = bacc.Bacc("TRN2", target_bir_lowering=False, debug=False,
                   num_devices=N_CORES)

    inp_d = nc.dram_tensor("inp", [PACK_TOTAL], dt.bfloat16,
                           kind="ExternalInput").ap()
    w1_d = inp_d[OFF_W1:OFF_MENTS].rearrange(
        "(p c h) -> p c h", p=128, h=HIDDEN)
    ments_d = inp_d[OFF_MENTS:OFF_W2B].rearrange(
        "(p k c) -> p k c", p=128, c=MENT_PAD)
    w2b_d = inp_d[OFF_W2B:OFF_MASKS].rearrange("(p c) -> p c", c=HC)
    masks_d = inp_d[OFF_MASKS:OFF_VMETA].rearrange(
        "(p o c) -> p o c", p=128, c=T)
    vmeta_d = inp_d[OFF_VMETA:PACK_TOTAL].rearrange(
        "(p v h) -> p v h", p=32, h=HIDDEN)
    out_d = nc.dram_tensor("out", [NT_ALL * T], dt.float32,
                           kind="ExternalOutput").ap()

    with tile.TileContext(nc) as tc:
        with (
            tc.tile_pool(name="const", bufs=1) as cpool,
            tc.tile_pool(name="tables", bufs=1) as tpool,
        ):
            w1_sb = cpool.tile([128, W1_CHUNKS, HIDDEN], dt.bfloat16)
            w2b = cpool.tile([128, HC], dt.bfloat16)

            u_sb = tpool.tile([128, M_CHUNKS * HIDDEN], dt.bfloat16)
            v_sb = tpool.tile([128, V_CHUNKS, HIDDEN], dt.bfloat16)

            nc.sync.dma_start(w2b[:], w2b_d[:])
            nc.sync.dma_start(w1_sb[:], w1_d[:])
            # W1c meta rows (replicated per V chunk) into v_sb rows 96..121;
            # rows 122..127 are never touched by a nonzero mask row.
            nc.sync.dma_start(v_sb[VROWS:VROWS + 32, :, :], vmeta_d[:])

            for _rep in range(reps):
              with (
                tc.tile_pool(name="mentT", bufs=1) as mtpool,
                tc.tile_pool(name="psA", bufs=4, space="PSUM") as psA,
              ):
                # three pieces of 6 mention-chunks each, so the first
                # projection matmuls start as soon as piece 0 lands
                mentT = []
                for piece in range(3):
                    mt = mtpool.tile([128, HC, 6 * 128], dt.bfloat16,
                                     tag=f"mt{piece}", name=f"mentT{piece}")
                    nc.sync.dma_start(
                        mt[:], ments_d[:, :, piece * 768:(piece + 1) * 768])
                    mentT.append(mt)

                # ---- U (18 chunks) and V (first 6 chunks) projections ----
                for r in range(M_CHUNKS if "p" in phases else 0):
                    u5 = psA.tile([128, 512], dt.float32, tag="p5")
                    u2 = psA.tile([128, 256], dt.float32, tag="p2")
                    do_v = r < V_CHUNKS
                    if do_v:
                        v5 = psA.tile([128, 512], dt.float32, tag="p5")
                        v2 = psA.tile([128, 256], dt.float32, tag="p2")
                    for k in range(HC):
                        lhs = mentT[:, k, ts(r, 128)]
                        st0, sp1 = (k == 0), (k == HC - 1)
                        nc.tensor.matmul(u5[:], lhs, w1_sb[:, k, :512],
                                         start=st0, stop=sp1)
                        nc.tensor.matmul(u2[:], lhs, w1_sb[:, k, 512:],
                                         start=st0, stop=sp1)
                        if do_v:
                            nc.tensor.matmul(v5[:], lhs, w1_sb[:, 6 + k, :512],
                                             start=st0, stop=sp1)
                            nc.tensor.matmul(v2[:], lhs, w1_sb[:, 6 + k, 512:],
                                             start=st0, stop=sp1)
                    ro = r * HIDDEN
                    nc.vector.tensor_copy(u_sb[:, ro:ro + 512], u5[:])
                    nc.vector.tensor_copy(u_sb[:, ro + 512:ro + HIDDEN], u2[:])
                    if do_v:
                        nc.scalar.copy(v_sb[:VROWS, r, :512], v5[:VROWS, :])
                        nc.scalar.copy(v_sb[:VROWS, r, 512:], v2[:VROWS, :])

            # ---- pair tiles: DMA one-hot planes + expand + relu + dot ----
              with (
                  tc.tile_pool(name="oh", bufs=3) as ohpool,
                  tc.tile_pool(name="h", bufs=4) as hpool,
                  tc.tile_pool(name="o", bufs=1) as opool,
                  tc.tile_pool(name="psD", bufs=2, space="PSUM") as psD,
                  tc.tile_pool(name="psL", bufs=2, space="PSUM") as psL,
              ):
                  relu = mybir.ActivationFunctionType.Relu
                  ident = mybir.ActivationFunctionType.Identity
                  lt_all = opool.tile([1, NT_ALL * T], dt.float32, tag="lt")
                  if "d" not in phases:
                      nc.vector.memset(lt_all[:], 0.0)
                  for t in range(NT_ALL if "d" in phases else 0):
                      oh_t = ohpool.tile([128, N_PLANES, T], dt.bfloat16,
                                         tag="oh")
                      nc.sync.dma_start(
                          oh_t[:],
                          masks_d[:, t * N_PLANES:(t + 1) * N_PLANES, :])
                      pl = psL.tile([1, T], dt.float32, tag="pl")
                      for g in range(2):
                          ph = psD.tile([128, 3, T], dt.float32, tag="ph")
                          for gi in range(3):
                              hc = 3 * g + gi
                              for ui, (c, lo, hi) in enumerate(URUNS[t]):
                                  lhs = u_sb[:, c * HIDDEN + hc * 128:
                                             c * HIDDEN + (hc + 1) * 128]
                                  nc.tensor.matmul(ph[:, gi, lo:hi], lhs,
                                                   oh_t[:, 0, lo:hi],
                                                   start=(ui == 0), stop=False)
                              nv = len(VRUNS[t])
                              for i, (j, lo, hi) in enumerate(VRUNS[t]):
                                  nc.tensor.matmul(ph[:, gi, lo:hi],
                                                   v_sb[:, j, ts(hc, 128)],
                                                   oh_t[:, 1, lo:hi],
                                                   start=False,
                                                   stop=(i == nv - 1))
                          h_sb = hpool.tile([128, 3, T], dt.bfloat16, tag="h")
                          if g == 0:
                              nc.scalar.activation(h_sb[:], ph[:], relu)
                          else:
                              nc.vector.tensor_scalar_max(h_sb[:], ph[:], 0.0)
                          for gi in range(3):
                              hc = 3 * g + gi
                              nc.tensor.matmul(pl[:], w2b[:, hc:hc + 1],
                                               h_sb[:, gi, :],
                                               start=(hc == 0),
                                               stop=(hc == HC - 1))
                      nc.scalar.activation(lt_all[:, ts(t, T)], pl[:], ident)
                  nc.sync.dma_start(
                      out_d.rearrange("(o c) -> o c", o=1), lt_all[:])

    nc.compile()
    return nc


def _get_compiled():
    global _COMPILED
    if _COMPILED is None:
        _COMPILED = _build()
    return _COMPILED


def make_in_maps(mention_reprs, coref_mention_pairs, coref_eds, ed_table,
                 W1, b1, W2, b2):
    import ml_dtypes

    bf16 = ml_dtypes.bfloat16
    mention_reprs = np.asarray(mention_reprs, dtype=np.float32)
    pairs = np.asarray(coref_mention_pairs).astype(np.int64)
    eds = np.asarray(coref_eds).astype(np.int64)
    W1 = np.asarray(W1, dtype=np.float32)
    W2 = np.asarray(W2, dtype=np.float32)
    b1 = np.asarray(b1, dtype=np.float32).reshape(HIDDEN)
    b2 = np.asarray(b2, dtype=np.float32)
    ed_table = np.asarray(ed_table, dtype=np.float32)

    w1p = np.zeros((W1_ROWS_PAD, HIDDEN), np.float32)
    w1p[:W1.shape[0]] = W1
    w1p[W1.shape[0]] = b1                      # b1 folded (row 1561)
    w2b = np.ascontiguousarray(W2.reshape(HC, 128).T)  # [p, c] = W2[c*128+p]

    w1_flat = np.ascontiguousarray(
        w1p.reshape(W1_CHUNKS, 128, HIDDEN).transpose(1, 0, 2)
    ).astype(bf16).reshape(-1)
    w2b_flat = w2b.astype(bf16).reshape(-1)
    edT = ed_table.T                           # [25, 300]
    w1c = w1p[1536:1536 + 26]                  # [26, 768] incl b1 row
    w1c32 = np.zeros((32, HIDDEN), np.float32)
    w1c32[:26] = w1c
    vmeta_flat = np.ascontiguousarray(
        np.broadcast_to(w1c32[:, None, :], (32, V_CHUNKS, HIDDEN))
    ).astype(bf16).reshape(-1)

    # permuted mention index -> padded mentT row (bucket: 6 chunks of 96)
    rowmap = np.empty(N_MENT, np.int64)
    g = np.arange(512)
    rowmap[:512] = 128 * (g // VROWS) + g % VROWS
    rowmap[512:] = 768 + np.arange(N_MENT - 512)

    lanes = np.arange(128, dtype=np.int64)

    in_maps = []
    placements = []
    for core in range(N_CORES):
        b = core // SLICES
        q = core % SLICES
        bucket = np.arange(512 * q, min(512 * (q + 1), N_MENT))
        rest = np.concatenate([np.arange(0, 512 * q),
                               np.arange(min(512 * (q + 1), N_MENT), N_MENT)])
        perm = np.concatenate([bucket, rest])
        inv_perm = np.empty(N_MENT, np.int64)
        inv_perm[perm] = np.arange(N_MENT)

        ments = np.zeros((MENT_PAD, HIDDEN), np.float32)
        ments[rowmap] = mention_reprs[b][perm]
        mentsT_flat = np.ascontiguousarray(
            ments.T.reshape(HC, 128, MENT_PAD).transpose(1, 0, 2)
        ).astype(bf16).reshape(-1)

        bsel = (pairs[b, :, 1] >= 512 * q) & (pairs[b, :, 1] < 512 * (q + 1))
        psel = np.nonzero(bsel)[0]
        a_row = rowmap[inv_perm[pairs[b, psel, 0]]]
        b_loc = inv_perm[pairs[b, psel, 1]]
        e_val = eds[b, psel]

        # strict cell assignment: pos = global column, -1 -> host fallback
        n = len(psel)
        pos = np.full(n, -1, np.int64)
        fill = np.zeros((M_CHUNKS, V_CHUNKS), np.int64)
        cs = a_row // 128
        js = b_loc // VROWS
        for i in range(n):
            c, j = cs[i], js[i]
            if fill[c, j] < CELLQ[c, j]:
                pos[i] = CELL_BASE[c, j] + fill[c, j]
                fill[c, j] += 1
        miss = pos < 0
        slop_logits = None
        if miss.any():
            sp = np.nonzero(miss)[0]
            cat = np.concatenate([
                mention_reprs[b][pairs[b, psel[sp], 0]],
                mention_reprs[b][pairs[b, psel[sp], 1]],
                ed_table[e_val[sp]],
            ], axis=1)
            h = np.maximum(cat @ W1 + b1, 0.0)
            slop_logits = (h @ W2).reshape(-1) + b2.reshape(-1)[0]

        ok = np.nonzero(~miss)[0]
        gcol = pos[ok]
        ucode = np.full(NT_ALL * T, 255, np.int64)
        vcode = np.full(NT_ALL * T, 255, np.int64)
        ucode[gcol] = a_row[ok] % 128
        vcode[gcol] = b_loc[ok] % VROWS

        # planes [128, NT*2, T]: per tile (U onehot, V onehot+meta)
        masksU = (ucode.reshape(NT_ALL, T)[None, :, :]
                  == lanes[:, None, None]).astype(bf16)
        vplane = np.zeros((128, NT_ALL * T), np.float32)
        vplane[:VROWS, :] = (vcode[None, :] == lanes[:VROWS, None])
        vplane[VROWS:VROWS + META, gcol] = edT[:, e_val[ok]]
        vplane[VROWS + META, gcol] = 1.0
        planes = np.empty((128, NT_ALL, N_PLANES, T), bf16)
        planes[:, :, 0, :] = masksU
        planes[:, :, 1, :] = vplane.reshape(128, NT_ALL, T).astype(bf16)

        placements.append((psel, b, pos, slop_logits))
        pack = np.empty(PACK_TOTAL, bf16)
        pack[OFF_W1:OFF_MENTS] = w1_flat
        pack[OFF_MENTS:OFF_W2B] = mentsT_flat
        pack[OFF_W2B:OFF_MASKS] = w2b_flat
        pack[OFF_MASKS:OFF_VMETA] = planes.reshape(-1)
        pack[OFF_VMETA:PACK_TOTAL] = vmeta_flat
        in_maps.append({"inp": pack})
    make_in_maps.placements = placements
    make_in_maps.b2 = float(b2.reshape(-1)[0])
    return in_maps


def unshard(results, placements):
    b2 = make_in_maps.b2
    out = np.zeros((B, N_PAIRS), np.float32)
    for core in range(N_CORES):
        psel, b, pos, slop_logits = placements[core]
        vals = results[core]["out"]
        ok = pos >= 0
        out[b, psel[ok]] = vals[pos[ok]] + b2
        if slop_logits is not None:
            out[b, psel[~ok]] = slop_logits
    return out


def kernel(**inputs):
    from concourse.bass_utils import run_bass_kernel_spmd

    nc = _get_compiled()
    in_maps = make_in_maps(**inputs)
    placements = make_in_maps.placements
    res = run_bass_kernel_spmd(nc, in_maps, list(range(N_CORES)))
    return unshard(res.results, placements)
